# revision 8
# baseline (speedup 1.0000x reference)
"""Bass/Trainium2 kernel for batched cross-attention (nn_Attention).

Reference math (per batch element, B=8 sharded one-per-core):
    tmp1   = h @ W_b                  [S, D]
    scores = tmp1 @ b^T               [S, S]
    attn   = softmax(scores, -1)
    cxt    = attn @ b                 [S, D]

Per-core layout (S=4096, D=128):
  - hT, bT  [D, S] f32 built via PE transposes of 128x128 tiles.
  - tmp1T   [D, S] = W_b (stationary) @ hT, in float32r (tf32-class) mode.
  - scoresT [s_tile(128p), t_block] = bT-tile (stationary) @ tmp1T, f32r.
    Softmax denominator is handled without a partition reduction: exp is
    taken with NO max subtraction (scores are ~N(0, 128); max |s| < 90 so
    fp32 exp cannot overflow), and the sum over s rides along the context
    matmul as an extra ones-column in the rhs.
  - attnT tiles [s(128p), t_block] bf16 from one big ACT exp per s-tile.
  - cxt[t_tile, 0:128] + denom[t_tile, 128] accumulate over the 32 s tiles
    in PSUM: lhsT = attnT slice, rhs = [b_bf16 | 1].
  - normalize on DVE (reciprocal + per-partition scalar mul), DMA out.
"""

import sys

if "/opt/trn_rl_repo" not in sys.path:
    sys.path.insert(0, "/opt/trn_rl_repo")

import numpy as np

B = 8
S = 4096
D = 128
P = 128
NT = S // P          # 32 seq tiles
TB = 1024            # t-block width (scores PSUM tensor = 2 banks)
NB = S // TB         # 4 t-blocks
QCHUNK = 512         # fp32 moving-operand max
SHIFT = 48.0         # exp(s - SHIFT): keeps exp finite (score max ~91)

_GRAPH = None


def _build_graph():
    import concourse.mybir as mybir
    import concourse.tile as tile
    from concourse import bacc
    from concourse.masks import make_identity

    f32 = mybir.dt.float32
    f32r = mybir.dt.float32r
    bf16 = mybir.dt.bfloat16
    Exp = mybir.ActivationFunctionType.Exp

    nc = bacc.Bacc()
    h_ext = nc.declare_dram_parameter("h", [S, D], f32, isOutput=False)
    b_ext = nc.declare_dram_parameter("b", [S, D], f32, isOutput=False)
    w_ext = nc.declare_dram_parameter("W_b", [D, D], f32, isOutput=False)
    out_ext = nc.declare_dram_parameter("out", [S, D], f32, isOutput=True)

    with tile.TileContext(nc) as tc:
        with (
            tc.tile_pool(name="const", bufs=1) as const_pool,
            tc.tile_pool(name="big", bufs=1) as big,
            tc.tile_pool(name="loads", bufs=6) as loads,
            tc.tile_pool(name="attn_pool", bufs=44) as attn_pool,
            tc.tile_pool(name="outp", bufs=4) as outp,
            tc.tile_pool(name="small", bufs=4) as small,
            tc.tile_pool(name="ps_small", bufs=2, space="PSUM") as ps_small,
            tc.tile_pool(name="ps_sc", bufs=2, space="PSUM") as ps_sc,
        ):
            ident = const_pool.tile([P, P], f32)
            make_identity(nc, ident)
            W_sb = const_pool.tile([D, D], f32)
            nc.sync.dma_start(out=W_sb, in_=w_ext[:, :])
            # fp32r matmul operands must be produced pre-rounded to fp32r
            W_r = const_pool.tile([D, D], f32r)
            nc.vector.tensor_copy(W_r, W_sb)
            shift_ap = const_pool.tile([P, 1], f32)
            nc.vector.memset(shift_ap, -SHIFT)

            hT = big.tile([P, S], f32r)
            bT = big.tile([P, S], f32r)
            t1T = big.tile([P, S], f32r)
            b1 = big.tile([P, NT, D + 1], bf16)   # [b | ones] per s-tile
            nc.vector.memset(b1[:, :, D : D + 1], 1.0)

            # --- b: transpose into bT (f32) and cast into b1 (bf16) ---
            for i in range(NT):
                b_t = loads.tile([P, D], f32, tag="ld")
                nc.sync.dma_start(out=b_t, in_=b_ext[i * P : (i + 1) * P, :])
                ps = ps_small.tile([P, QCHUNK], f32, tag="pst", bufs=2)
                nc.tensor.transpose(ps[:, 0:P], b_t, ident)
                # alternate copy engine so prologue copies run on both DVE/ACT
                if i % 2 == 0:
                    nc.vector.tensor_copy(bT[:, i * P : (i + 1) * P], ps[:, 0:P])
                else:
                    nc.scalar.copy(bT[:, i * P : (i + 1) * P], ps[:, 0:P])
                nc.vector.tensor_copy(b1[:, i, 0:D], b_t)

            # --- h: transpose into hT (f32) ---
            for i in range(NT):
                h_t = loads.tile([P, D], f32, tag="ld")
                nc.sync.dma_start(out=h_t, in_=h_ext[i * P : (i + 1) * P, :])
                ps = ps_small.tile([P, QCHUNK], f32, tag="pst", bufs=2)
                nc.tensor.transpose(ps[:, 0:P], h_t, ident)
                if i % 2 == 0:
                    nc.vector.tensor_copy(hT[:, i * P : (i + 1) * P], ps[:, 0:P])
                else:
                    nc.scalar.copy(hT[:, i * P : (i + 1) * P], ps[:, 0:P])

            # --- tmp1T = W_b^T-applied queries: [e, t] ---
            for c in range(S // QCHUNK):
                ps = ps_small.tile([P, QCHUNK], f32, tag="pst", bufs=2)
                nc.tensor.matmul(
                    ps,
                    lhsT=W_r,
                    rhs=hT[:, c * QCHUNK : (c + 1) * QCHUNK],
                    start=True,
                    stop=True,
                )
                if c % 2 == 0:
                    nc.vector.tensor_copy(t1T[:, c * QCHUNK : (c + 1) * QCHUNK], ps)
                else:
                    nc.scalar.copy(t1T[:, c * QCHUNK : (c + 1) * QCHUNK], ps)

            # --- main loop over t-blocks ---
            for tb in range(NB):
                attn_tiles = []
                for si in range(NT):
                    ps_s = ps_sc.tile([P, TB], f32, tag="sc")
                    for c in range(TB // QCHUNK):
                        nc.tensor.matmul(
                            ps_s[:, c * QCHUNK : (c + 1) * QCHUNK],
                            lhsT=bT[:, si * P : (si + 1) * P],
                            rhs=t1T[
                                :, tb * TB + c * QCHUNK : tb * TB + (c + 1) * QCHUNK
                            ],
                            start=True,
                            stop=True,
                        )
                    at = attn_pool.tile([P, TB], bf16, tag="attn")
                    nc.scalar.activation(out=at, in_=ps_s, func=Exp, bias=shift_ap)
                    attn_tiles.append(at)

                for tt in range(TB // P):
                    ps_c = ps_small.tile([P, D + 1], f32, tag="psc", bufs=2)
                    for si in range(NT):
                        nc.tensor.matmul(
                            ps_c,
                            lhsT=attn_tiles[si][:, tt * P : (tt + 1) * P],
                            rhs=b1[:, si, :],
                            start=(si == 0),
                            stop=(si == NT - 1),
                        )
                    recip = small.tile([P, 1], f32, tag="recip")
                    nc.vector.reciprocal(recip, ps_c[:, D : D + 1])
                    o_t = outp.tile([P, D], f32, tag="ot")
                    nc.vector.tensor_scalar_mul(o_t, ps_c[:, 0:D], recip)
                    row0 = (tb * TB // P + tt) * P
                    nc.sync.dma_start(out=out_ext[row0 : row0 + P, :], in_=o_t)

    return nc


def _get_graph():
    global _GRAPH
    if _GRAPH is None:
        _GRAPH = _build_graph()
        _GRAPH.finalize()
    return _GRAPH


def kernel(b, h, W_b, **_ignored):
    nc = _get_graph()
    from concourse.bass_utils import run_bass_kernel_spmd

    b = np.asarray(b, dtype=np.float32)
    h = np.asarray(h, dtype=np.float32)
    W_b = np.asarray(W_b, dtype=np.float32)
    in_maps = [
        {
            "b": np.ascontiguousarray(b[i]),
            "h": np.ascontiguousarray(h[i]),
            "W_b": np.ascontiguousarray(W_b),
        }
        for i in range(B)
    ]
    res = run_bass_kernel_spmd(nc, in_maps, core_ids=list(range(B)))
    return np.stack([res.results[i]["out"] for i in range(B)], axis=0)


# revision 10
# speedup vs baseline: 1.0975x; 1.0975x over previous
"""Bass/Trainium2 kernel for batched cross-attention (nn_Attention).

Reference math (per batch element, B=8 sharded one-per-core):
    tmp1   = h @ W_b                  [S, D]
    scores = tmp1 @ b^T               [S, S]
    attn   = softmax(scores, -1)
    cxt    = attn @ b                 [S, D]

Per-core schedule (S=4096, D=128), v2 — interleaved:
  - h, b loaded with 4 big chunked DMAs each (partition-major rearrange),
    PE-transposed tile-by-tile into hT/bT [D, S] fp32r; tmp1T = W_b^T @ hT.
  - main loop over 4 t-blocks of 1024, inner loop over 32 s-tiles:
      QK: scoresT[s_tile, t_block] = bT-tile^T @ tmp1T (fp32r, 2x512 chunks)
      exp: one ACT instruction [128, 1024] PSUM->SBUF bf16, bias=-SHIFT
           (softmax is shift-invariant; score max ~91 would overflow fp32)
      cxt: 8 accumulating matmuls (one per t-tile of 128) into packed PSUM
           accumulators [cxt | denom] = attnT-slice^T @ [b_bf16 | 1].
    The denominator rides along as a ones-column; no partition reduction.
  - per block: 8 normalizes (DVE reciprocal + per-partition scalar mul)
    into a staging tile, one 512KB output DMA.
"""

import sys

if "/opt/trn_rl_repo" not in sys.path:
    sys.path.insert(0, "/opt/trn_rl_repo")

import numpy as np

B = 8
S = 4096
D = 128
P = 128
NT = S // P          # 32 seq tiles
TB = 1024            # t-block width
NB = S // TB         # 4 t-blocks
TT = TB // P         # 8 t-tiles per block
QCHUNK = 512         # fp32 moving-operand max
SHIFT = 48.0         # exp(s - SHIFT): keeps exp finite (score max ~91)
ACC_PACK = 3         # [128,129] accumulators packed per PSUM bank

_GRAPH = None


def _build_graph():
    import concourse.mybir as mybir
    import concourse.tile as tile
    from concourse import bacc
    from concourse.masks import make_identity

    f32 = mybir.dt.float32
    f32r = mybir.dt.float32r
    bf16 = mybir.dt.bfloat16
    Exp = mybir.ActivationFunctionType.Exp

    nc = bacc.Bacc()
    h_ext = nc.declare_dram_parameter("h", [S, D], f32, isOutput=False)
    b_ext = nc.declare_dram_parameter("b", [S, D], f32, isOutput=False)
    w_ext = nc.declare_dram_parameter("W_b", [D, D], f32, isOutput=False)
    out_ext = nc.declare_dram_parameter("out", [S, D], f32, isOutput=True)

    h_pnd = h_ext.rearrange("(n p) d -> p n d", p=P)   # [128, 32, 128]
    b_pnd = b_ext.rearrange("(n p) d -> p n d", p=P)
    out_pnd = out_ext.rearrange("(n p) d -> p n d", p=P)

    n_acc_tiles = (TT + ACC_PACK - 1) // ACC_PACK      # 3

    with tile.TileContext(nc) as tc:
        with (
            tc.tile_pool(name="const", bufs=1) as const_pool,
            tc.tile_pool(name="big", bufs=1) as big,
            tc.tile_pool(name="attn_pool", bufs=4) as attn_pool,
            tc.tile_pool(name="outp", bufs=2) as outp,
            tc.tile_pool(name="small", bufs=4) as small,
            tc.tile_pool(name="ps_sc", bufs=2, space="PSUM") as ps_sc,
            tc.tile_pool(name="ps_acc", bufs=1, space="PSUM") as ps_acc,
        ):
            ident = const_pool.tile([P, P], f32)
            make_identity(nc, ident)
            W_sb = const_pool.tile([D, D], f32)
            nc.sync.dma_start(out=W_sb, in_=w_ext[:, :])
            # fp32r matmul operands must be produced pre-rounded to fp32r
            W_r = const_pool.tile([D, D], f32r)
            nc.vector.tensor_copy(W_r, W_sb)
            shift_ap = const_pool.tile([P, 1], f32)
            nc.vector.memset(shift_ap, -SHIFT)

            h_sb = big.tile([P, NT, D], f32)
            b_sb = big.tile([P, NT, D], f32)
            NCH = 4
            for c in range(NCH):
                sl = slice(c * NT // NCH, (c + 1) * NT // NCH)
                nc.sync.dma_start(out=b_sb[:, sl, :], in_=b_pnd[:, sl, :])
            for c in range(NCH):
                sl = slice(c * NT // NCH, (c + 1) * NT // NCH)
                nc.sync.dma_start(out=h_sb[:, sl, :], in_=h_pnd[:, sl, :])

            hT = big.tile([P, S], f32r)
            bT = big.tile([P, S], f32r)
            t1T = big.tile([P, S], f32r)
            b1 = big.tile([P, NT, D + 1], bf16)   # [b | ones] per s-tile
            nc.vector.memset(b1[:, :, D : D + 1], 1.0)

            # --- b: transpose into bT (f32r) and cast into b1 (bf16) ---
            for i in range(NT):
                ps = ps_sc.tile([P, TB], f32, tag="sc")
                nc.tensor.transpose(ps[:, 0:P], b_sb[:, i, :], ident)
                if i % 2 == 0:
                    nc.vector.tensor_copy(bT[:, i * P : (i + 1) * P], ps[:, 0:P])
                else:
                    nc.scalar.copy(bT[:, i * P : (i + 1) * P], ps[:, 0:P])
                nc.vector.tensor_copy(b1[:, i, 0:D], b_sb[:, i, :])

            # --- h: transpose into hT; tmp1T = W_b^T @ hT per 512-chunk ---
            for i in range(NT):
                ps = ps_sc.tile([P, TB], f32, tag="sc")
                nc.tensor.transpose(ps[:, 0:P], h_sb[:, i, :], ident)
                if i % 2 == 0:
                    nc.vector.tensor_copy(hT[:, i * P : (i + 1) * P], ps[:, 0:P])
                else:
                    nc.scalar.copy(hT[:, i * P : (i + 1) * P], ps[:, 0:P])
            for c in range(S // QCHUNK):
                ps = ps_sc.tile([P, TB], f32, tag="sc")
                nc.tensor.matmul(
                    ps[:, 0:QCHUNK],
                    lhsT=W_r,
                    rhs=hT[:, c * QCHUNK : (c + 1) * QCHUNK],
                    start=True,
                    stop=True,
                )
                if c % 2 == 0:
                    nc.vector.tensor_copy(t1T[:, c * QCHUNK : (c + 1) * QCHUNK], ps[:, 0:QCHUNK])
                else:
                    nc.scalar.copy(t1T[:, c * QCHUNK : (c + 1) * QCHUNK], ps[:, 0:QCHUNK])

            # --- main loop: QK + exp + interleaved cxt accumulation ---
            for tb in range(NB):
                accs = []
                for a in range(n_acc_tiles):
                    w = min(ACC_PACK, TT - a * ACC_PACK) * (D + 1)
                    acc = ps_acc.tile([P, ACC_PACK * (D + 1)], f32, tag=f"acc{a}")
                    accs.append(acc)

                for si in range(NT):
                    ps_s = ps_sc.tile([P, TB], f32, tag="sc")
                    for c in range(TB // QCHUNK):
                        nc.tensor.matmul(
                            ps_s[:, c * QCHUNK : (c + 1) * QCHUNK],
                            lhsT=bT[:, si * P : (si + 1) * P],
                            rhs=t1T[
                                :, tb * TB + c * QCHUNK : tb * TB + (c + 1) * QCHUNK
                            ],
                            start=True,
                            stop=True,
                        )
                    at = attn_pool.tile([P, TB], bf16, tag="attn")
                    nc.scalar.activation(out=at, in_=ps_s, func=Exp, bias=shift_ap)
                    for tt in range(TT):
                        acc = accs[tt // ACC_PACK]
                        off = (tt % ACC_PACK) * (D + 1)
                        # start=True marks the WHOLE 2KB psum bank pending-zero,
                        # so it must be issued exactly once per bank (first
                        # region, first s-tile); later first-writes to the other
                        # packed regions land on still-pending-zero bytes and
                        # overwrite, then accumulate.
                        nc.tensor.matmul(
                            acc[:, off : off + D + 1],
                            lhsT=at[:, tt * P : (tt + 1) * P],
                            rhs=b1[:, si, :],
                            start=(si == 0 and tt % ACC_PACK == 0),
                            stop=(si == NT - 1),
                            skip_group_check=True,
                        )

                o_big = outp.tile([P, TT, D], f32, tag="ot")
                for tt in range(TT):
                    acc = accs[tt // ACC_PACK]
                    off = (tt % ACC_PACK) * (D + 1)
                    recip = small.tile([P, 1], f32, tag="recip")
                    nc.vector.reciprocal(recip, acc[:, off + D : off + D + 1])
                    nc.vector.tensor_scalar_mul(
                        o_big[:, tt, :], acc[:, off : off + D], recip
                    )
                nc.sync.dma_start(
                    out=out_pnd[:, tb * TT : (tb + 1) * TT, :], in_=o_big
                )

    return nc


def _get_graph():
    global _GRAPH
    if _GRAPH is None:
        _GRAPH = _build_graph()
        _GRAPH.finalize()
    return _GRAPH


def kernel(b, h, W_b, **_ignored):
    nc = _get_graph()
    from concourse.bass_utils import run_bass_kernel_spmd

    b = np.asarray(b, dtype=np.float32)
    h = np.asarray(h, dtype=np.float32)
    W_b = np.asarray(W_b, dtype=np.float32)
    in_maps = [
        {
            "b": np.ascontiguousarray(b[i]),
            "h": np.ascontiguousarray(h[i]),
            "W_b": np.ascontiguousarray(W_b),
        }
        for i in range(B)
    ]
    res = run_bass_kernel_spmd(nc, in_maps, core_ids=list(range(B)))
    return np.stack([res.results[i]["out"] for i in range(B)], axis=0)
